# revision 8
# baseline (speedup 1.0000x reference)
"""Bass/Trainium2 kernel for nn_LocalAggregator (GNN message passing).

Math per batch b (hidden [64,128], adj [64,64] in {0..4}, a [4,128]):
    e_k[i,j] = leakyrelu_{0.2}( sum_d hidden[i,d]*hidden[j,d]*a[k,d] )
    alpha    = softmax_j( where(adj==k+1, e_k, -9e15) )
    out      = alpha @ hidden

Device strategy (8 cores, 64 batches/core, OCT = 8 batches/iter):
  - e_k is SYMMETRIC in (i,j): masking with the host-TRANSPOSED
    adjacency yields transposed attention weights directly.
  - w_all[d,(k,l,j)] = hT * a_k precomputed on HOST (memory-bound
    regime: ship it, don't burn vector cycles).
  - ONE fused input DMA per oct; the one-hot additive mask ships as
    fp8 {0,-192} bytes inside the bf16 tensor (bitcast view on SBUF)
    and is ADDED into the e-PSUM by an fp8 identity matmul
    (start=True) before the e-matmuls accumulate on top.  The per-k
    select then becomes a max-fold:
      z_sel = max_k (lrelu(e_k + m_k));  masked entries stay <= -38
    so exp(z) ~ 0 there.  This kills the exp/select/mul/sum chain:
      ACT per gp: one Prelu [128,512]->fp16, one Exp [128,128]
      DVE per gp: two max-folds;  one strided output evac per oct
  - everything runs per GP-half (4 batches) on 1-bank PSUM tiles for
    deep PE/ACT/DVE overlap; ones-column in hh makes the out-matmul
    emit the softmax denominator; normalization on HOST.
"""

import numpy as np
import ml_dtypes

from contextlib import ExitStack

import concourse.bass as bass
import concourse.tile as tile
from concourse import bacc, mybir
from concourse._compat import with_exitstack
from concourse.bass_utils import run_bass_kernel_spmd

BF16 = mybir.dt.bfloat16
FP16 = mybir.dt.float16
FP8 = mybir.dt.float8e4
F32 = mybir.dt.float32
ALU = mybir.AluOpType
ACTF = mybir.ActivationFunctionType

B, N, D, K = 512, 64, 128, 4
NCORES = 8
BPC = B // NCORES          # 64 batches per core
OCTS = BPC // 8            # 8 octs of 8 batches per core
HHW = 132                  # hidden cols + ones col + pad
MASK = -192.0              # additive mask; exact in fp8e4m3, exp() -> ~0
CW = 512 + 2048 + 528 + 512  # hT8 | wall8 | hh8 | indm(fp8 as bf16) = 3600
OWC = 4 * 129              # out tile cols: (num 128 | denom) x (gp,t)


@with_exitstack
def _kernel_body(ctx, tc, cmb2_d, id_d, out_d):
    nc = tc.nc

    const_pool = ctx.enter_context(tc.tile_pool(name="const", bufs=1))
    in_pool = ctx.enter_context(tc.tile_pool(name="inp", bufs=3))
    work_pool = ctx.enter_context(tc.tile_pool(name="work", bufs=4))
    wal_pool = ctx.enter_context(tc.tile_pool(name="walp", bufs=6))
    psum_pool = ctx.enter_context(tc.tile_pool(name="psum", bufs=3, space="PSUM"))
    opsum_pool = ctx.enter_context(tc.tile_pool(name="opsum", bufs=2, space="PSUM"))
    scr_pool = ctx.enter_context(tc.tile_pool(name="scr", bufs=1, space="PSUM"))
    out_pool = ctx.enter_context(tc.tile_pool(name="outp", bufs=4))

    ident = const_pool.tile([128, 128], FP8, tag="ident")
    nc.sync.dma_start(out=ident[:], in_=id_d)

    # PE keep-warm: the tensor engine only reaches full clock after ~3us
    # of gap-free execution.  A scratch stream with no input deps bridges
    # the startup (DMA latency) and the per-oct gaps so the real matmuls
    # run at full rate.  Results are never read.
    fsrc = const_pool.tile([128, 512], FP8, tag="fsrc")
    nc.gpsimd.memset(fsrc[:], 0)
    scr = scr_pool.tile([128, 512], F32, tag="scr")

    def filler(n):
        for _ in range(n):
            nc.tensor.matmul(scr[:], lhsT=fsrc[:, 0:128], rhs=fsrc[:],
                             start=True, stop=True)

    filler(16)

    def out_block(g, wals, hh8):
        """out matmuls + evac + output DMA for oct g (runs 1 oct behind
        the e-chain so the PE never waits on the ACT/DVE chain)."""
        ops = opsum_pool.tile([128, 1024], F32, tag="ops")
        for gp in range(2):
            for l in range(4):
                t, u = l // 2, l % 2
                nc.tensor.matmul(
                    ops[u * 64:(u + 1) * 64,
                        gp * 512 + t * HHW: gp * 512 + (t + 1) * HHW],
                    lhsT=wals[gp][u * 64:(u + 1) * 64, t * 64:(t + 1) * 64],
                    rhs=hh8[u * 64:(u + 1) * 64,
                            gp * 264 + t * HHW: gp * 264 + (t + 1) * HHW],
                    start=True, stop=True,
                    tile_position=(u * 64, u * 64))
        # compact evac (num|den only), alternating DVE/ACT for balance
        osb = out_pool.tile([128, OWC], BF16, tag="osb")
        src = ops[:].rearrange("p (g q) -> p g q", g=2)[:, :, 0:264]
        src = src.rearrange("p g (t c) -> p g t c", t=2)[:, :, :, 0:129]
        dst = osb[:].rearrange("p (g t c) -> p g t c", g=2, t=2)
        if g % 2 == 0:
            nc.vector.tensor_copy(dst, src)
        else:
            nc.scalar.activation(dst, src, ACTF.Copy)
        nc.gpsimd.dma_start(out=out_d[g], in_=osb[:])

    prev = None
    for g in range(OCTS):
        # fused DOUBLE-oct load (fewer DMA-engine boundary stalls), per
        # oct: 0:512 hT8 [d,(g',l,i)] | 512:2560 wall8 [d,(g',k,l,j)] |
        # 2560:3088 hh8 [(u,j),(g',t,c)] | 3088:3600 fp8 mask bytes
        # [(u,x),(k,g',t,y)]
        if g % 2 == 0:
            cmb2 = in_pool.tile([128, 2 * CW], BF16, tag="cmb")
            nc.sync.dma_start(out=cmb2[:], in_=cmb2_d[g // 2])
        cmb = cmb2[:, (g % 2) * CW:(g % 2 + 1) * CW]

        # lagged out-block first: its inputs are long since ready, so the
        # PE stays busy while this oct's DMA lands
        if prev is not None:
            out_block(*prev)

        wallv = cmb[:, 512:2560].rearrange("p (g k l j) -> p g k l j",
                                           g=2, k=4, l=4)
        hh8 = cmb[:, 2560:3088]
        im8v = cmb[:, 3088:CW].bitcast(FP8).rearrange(
            "p (k g ty) -> p k g ty", k=4, g=2)

        wals = []
        for gp in range(2):
            # ---- e4[(u,x), (k,t,y)] = e_k[x,y] + mask (1-bank tile) ----
            e4 = psum_pool.tile([128, 512], F32, tag="e4")
            e4v = e4[:].rearrange("p (k t y) -> p k t y", k=4, t=2)
            # mask lands first (identity matmul, start=True resets bank)
            nc.tensor.matmul(
                e4[:].rearrange("p (k ty) -> p k ty", k=4),
                lhsT=ident[:],
                rhs=im8v[:, :, gp, :],
                start=True, stop=False)
            for l in range(4):
                t, u = l // 2, l % 2
                nc.tensor.matmul(
                    e4v[u * 64:(u + 1) * 64, :, t, :],
                    lhsT=cmb[:, gp * 256 + l * 64: gp * 256 + (l + 1) * 64],
                    rhs=wallv[:, gp, :, l, :],
                    start=False, stop=True,
                    tile_position=(0, u * 64))

            # ---- per-k select: ONE max-reduce over the k axis (PSUM) ----
            z = work_pool.tile([128, 128], FP16, tag="z")
            nc.vector.tensor_reduce(
                z[:],
                e4[:].rearrange("p (k ty) -> p ty k", k=4),
                mybir.AxisListType.X, ALU.max)

            # ---- leakyrelu (masked entries stay <= -38) then exp ----
            pz = work_pool.tile([128, 128], FP16, tag="pz")
            nc.scalar.activation(pz[:], z[:], ACTF.Prelu, alpha=0.2)
            wal = wal_pool.tile([128, 128], BF16, tag="wal")
            nc.scalar.activation(wal[:], pz[:], ACTF.Exp)
            wals.append(wal)

        # bridge the oct-boundary PE gap to hold the clock at full speed
        filler(3)
        prev = (g, wals, hh8)

    out_block(*prev)


def build_nc():
    nc = bacc.Bacc("TRN2", target_bir_lowering=False, debug=False)
    cmb2_d = nc.dram_tensor("cmb", [OCTS // 2, 128, 2 * CW], BF16,
                            kind="ExternalInput").ap()
    id_d = nc.dram_tensor("ident", [128, 128], FP8,
                          kind="ExternalInput").ap()
    out_d = nc.dram_tensor("out", [OCTS, 128, OWC], BF16,
                           kind="ExternalOutput").ap()
    with tile.TileContext(nc) as tc:
        _kernel_body(tc, cmb2_d, id_d, out_d)
    nc.compile()
    return nc


def _octify(x):
    """[B//4, 128, W] -> [B//8, 128, 2*W] pairing consecutive quads."""
    q, p, w = x.shape
    return (x.reshape(q // 2, 2, p, w).transpose(0, 2, 1, 3)
            .reshape(q // 2, p, 2 * w))


def prep_inputs(hidden, adj, a):
    """Host-side packing: bf16/fp8 casts, fused transposed layouts."""
    bf = ml_dtypes.bfloat16
    f8 = ml_dtypes.float8_e4m3
    hidden = np.asarray(hidden, dtype=np.float32)
    adj = np.asarray(adj)
    a = np.asarray(a, dtype=np.float32)

    hb = hidden.astype(bf)                                   # [B, 64, 128]

    # hT_q[q, d, l*64+i] = hidden[4q+l, i, d]
    hTf = (hidden.transpose(0, 2, 1)
           .reshape(B // 4, 4, D, N)
           .transpose(0, 2, 1, 3)
           .reshape(B // 4, D, 4 * N))
    hT = hTf.astype(bf)

    # wall_q[q, d, k*256+l*64+j] = hidden[4q+l, j, d] * a[k, d]
    wall = (hTf[:, None, :, :] * a[None, :, :, None]).astype(bf)
    wall = (wall.transpose(0, 2, 1, 3)
            .reshape(B // 4, D, 4 * 4 * N))

    # hh_q[q, u*64+j, t*HHW + c] : hidden rows + ones col for batch 4q+2t+u
    hh = np.zeros((B, N, HHW), dtype=bf)
    hh[:, :, 0:D] = hb
    hh[:, :, D] = bf(1.0)
    hhq = (hh.reshape(B // 4, 2, 2, N, HHW)
           .transpose(0, 2, 3, 1, 4)
           .reshape(B // 4, 2 * N, 2 * HHW))

    # indm[oct, u*64+x, k*256+g'*128+t*64+y] = 0 if adj[b][y,x]==k+1 else MASK
    # with b = oct*8 + g'*4 + t*2 + u; shipped as raw fp8 bytes inside cmb
    adjT = adj.transpose(0, 2, 1)                            # [b, x, y]
    mk = np.where(
        adjT[:, None, :, :] == np.arange(1, 5)[None, :, None, None],
        np.float32(0.0), np.float32(MASK)).astype(f8)        # [b, k, x, y]
    mk = mk.reshape(B // 8, 2, 2, 2, K, N, N)                # [o,g',t,u,k,x,y]
    indm = np.ascontiguousarray(
        mk.transpose(0, 3, 5, 4, 1, 2, 6).reshape(B // 8, 128, 1024))

    cmb16 = np.concatenate([_octify(hT), _octify(wall), _octify(hhq)], axis=2)
    cmb = np.concatenate(
        [cmb16.view(np.uint8), indm.view(np.uint8)], axis=2).view(bf)
    # pair consecutive octs into one row for double-oct DMAs
    cmb = (cmb.reshape(B // 16, 2, 128, CW).transpose(0, 2, 1, 3)
           .reshape(B // 16, 128, 2 * CW))
    cmb = np.ascontiguousarray(cmb)                          # [B//16,128,2CW]

    ident = np.ascontiguousarray(np.eye(128, dtype=f8))

    in_maps = []
    for c in range(NCORES):
        gsl = slice(c * OCTS // 2, (c + 1) * OCTS // 2)
        in_maps.append({"cmb": np.ascontiguousarray(cmb[gsl]),
                        "ident": ident})
    return in_maps


_NC_CACHE = {}


def run_device(hidden, adj, a, **spmd_kwargs):
    if "nc" not in _NC_CACHE:
        _NC_CACHE["nc"] = build_nc()
    nc = _NC_CACHE["nc"]
    in_maps = prep_inputs(hidden, adj, a)
    res = run_bass_kernel_spmd(nc, in_maps, list(range(NCORES)), **spmd_kwargs)
    # res[c]["out"]: [OCTS, 128, OWC]; [g, u*64+i, (gp,t)*129 + c]
    full = np.concatenate([res.results[c]["out"] for c in range(NCORES)],
                          axis=0)
    full = full.astype(np.float32)
    full = full.reshape(B // 8, 2, N, 2, 2, 129)             # [g, u, i, gp, t, c]
    num = full[..., 0:D]
    den = full[..., D:D + 1]
    outq = (num / den).transpose(0, 3, 4, 1, 2, 5)           # [g, gp, t, u, i, d]
    out = np.ascontiguousarray(outq.reshape(B, N, D))
    return out.astype(np.float32), res


def kernel(hidden, adj, a):
    out, _ = run_device(hidden, adj, a)
    return out


# revision 10
# speedup vs baseline: 1.0061x; 1.0061x over previous
"""Bass/Trainium2 kernel for nn_LocalAggregator (GNN message passing).

Math per batch b (hidden [64,128], adj [64,64] in {0..4}, a [4,128]):
    e_k[i,j] = leakyrelu_{0.2}( sum_d hidden[i,d]*hidden[j,d]*a[k,d] )
    alpha    = softmax_j( where(adj==k+1, e_k, -9e15) )
    out      = alpha @ hidden

Device strategy (8 cores, 64 batches/core, OCT = 8 batches/iter):
  - e_k is SYMMETRIC in (i,j): masking with the host-TRANSPOSED
    adjacency yields transposed attention weights directly.
  - w_all[d,(k,l,j)] = hT * a_k precomputed on HOST (memory-bound
    regime: ship it, don't burn vector cycles).
  - ONE fused input DMA per oct; the one-hot additive mask ships as
    fp8 {0,-192} bytes inside the bf16 tensor (bitcast view on SBUF)
    and is ADDED into the e-PSUM by an fp8 identity matmul
    (start=True) before the e-matmuls accumulate on top.  The per-k
    select then becomes a max-fold:
      z_sel = max_k (lrelu(e_k + m_k));  masked entries stay <= -38
    so exp(z) ~ 0 there.  This kills the exp/select/mul/sum chain:
      ACT per gp: one Prelu [128,512]->fp16, one Exp [128,128]
      DVE per gp: two max-folds;  one strided output evac per oct
  - everything runs per GP-half (4 batches) on 1-bank PSUM tiles for
    deep PE/ACT/DVE overlap; ones-column in hh makes the out-matmul
    emit the softmax denominator; normalization on HOST.
"""

import numpy as np
import ml_dtypes

from contextlib import ExitStack

import concourse.bass as bass
import concourse.tile as tile
from concourse import bacc, mybir
from concourse._compat import with_exitstack
from concourse.bass_utils import run_bass_kernel_spmd

BF16 = mybir.dt.bfloat16
FP16 = mybir.dt.float16
FP8 = mybir.dt.float8e4
F32 = mybir.dt.float32
ALU = mybir.AluOpType
ACTF = mybir.ActivationFunctionType

B, N, D, K = 512, 64, 128, 4
NCORES = 8
BPC = B // NCORES          # 64 batches per core
OCTS = BPC // 8            # 8 octs of 8 batches per core
HHW = 132                  # hidden cols + ones col + pad
MASK = -192.0              # additive mask; exact in fp8e4m3, exp() -> ~0
CW = 512 + 2048 + 528 + 512  # hT8 | wall8 | hh8 | indm(fp8 as bf16) = 3600
OWC = 4 * 129              # out tile cols: (num 128 | denom) x (gp,t)


@with_exitstack
def _kernel_body(ctx, tc, cmb_d, id_d, out_d):
    nc = tc.nc

    const_pool = ctx.enter_context(tc.tile_pool(name="const", bufs=1))
    in_pool = ctx.enter_context(tc.tile_pool(name="inp", bufs=6))
    work_pool = ctx.enter_context(tc.tile_pool(name="work", bufs=4))
    wal_pool = ctx.enter_context(tc.tile_pool(name="walp", bufs=6))
    psum_pool = ctx.enter_context(tc.tile_pool(name="psum", bufs=3, space="PSUM"))
    opsum_pool = ctx.enter_context(tc.tile_pool(name="opsum", bufs=2, space="PSUM"))
    scr_pool = ctx.enter_context(tc.tile_pool(name="scr", bufs=1, space="PSUM"))
    out_pool = ctx.enter_context(tc.tile_pool(name="outp", bufs=4))

    ident = const_pool.tile([128, 128], FP8, tag="ident")
    nc.sync.dma_start(out=ident[:], in_=id_d)

    # PE keep-warm: the tensor engine only reaches full clock after ~3us
    # of gap-free execution.  A scratch stream with no input deps bridges
    # the startup (DMA latency) and the per-oct gaps so the real matmuls
    # run at full rate.  Results are never read.
    fsrc = const_pool.tile([128, 512], FP8, tag="fsrc")
    nc.gpsimd.memset(fsrc[:], 0)
    scr = scr_pool.tile([128, 512], F32, tag="scr")

    def filler(n):
        for _ in range(n):
            nc.tensor.matmul(scr[:], lhsT=fsrc[:, 0:128], rhs=fsrc[:],
                             start=True, stop=True)

    filler(16)

    def out_block(g, wals, hh8):
        """out matmuls + evac + output DMA for oct g (runs 1 oct behind
        the e-chain so the PE never waits on the ACT/DVE chain)."""
        ops = opsum_pool.tile([128, 1024], F32, tag="ops")
        for gp in range(2):
            for l in range(4):
                t, u = l // 2, l % 2
                nc.tensor.matmul(
                    ops[u * 64:(u + 1) * 64,
                        gp * 512 + t * HHW: gp * 512 + (t + 1) * HHW],
                    lhsT=wals[gp][u * 64:(u + 1) * 64, t * 64:(t + 1) * 64],
                    rhs=hh8[u * 64:(u + 1) * 64,
                            gp * 264 + t * HHW: gp * 264 + (t + 1) * HHW],
                    start=True, stop=True,
                    tile_position=(u * 64, u * 64))
        # compact evac (num|den only), alternating DVE/ACT for balance
        osb = out_pool.tile([128, OWC], BF16, tag="osb")
        src = ops[:].rearrange("p (g q) -> p g q", g=2)[:, :, 0:264]
        src = src.rearrange("p g (t c) -> p g t c", t=2)[:, :, :, 0:129]
        dst = osb[:].rearrange("p (g t c) -> p g t c", g=2, t=2)
        if g % 2 == 0:
            nc.vector.tensor_copy(dst, src)
        else:
            nc.scalar.activation(dst, src, ACTF.Copy)
        nc.gpsimd.dma_start(out=out_d[g], in_=osb[:])

    prev = None
    for g in range(OCTS):
        # fused oct load: 0:512 hT8 [d,(g',l,i)] | 512:2560 wall8
        # [d,(g',k,l,j)] | 2560:3088 hh8 [(u,j),(g',t,c)] |
        # 3088:3600 fp8 mask bytes [(u,x),(k,g',t,y)]
        cmb = in_pool.tile([128, CW], BF16, tag="cmb")
        nc.sync.dma_start(out=cmb[:, 0:2560], in_=cmb_d[g][:, 0:2560])
        nc.sync.dma_start(out=cmb[:, 2560:CW], in_=cmb_d[g][:, 2560:CW])

        # lagged out-block first: its inputs are long since ready, so the
        # PE stays busy while this oct's DMA lands
        if prev is not None:
            out_block(*prev)

        wallv = cmb[:, 512:2560].rearrange("p (g k l j) -> p g k l j",
                                           g=2, k=4, l=4)
        hh8 = cmb[:, 2560:3088]
        im8v = cmb[:, 3088:CW].bitcast(FP8).rearrange(
            "p (k g ty) -> p k g ty", k=4, g=2)

        wals = []
        for gp in range(2):
            # ---- e4[(u,x), (k,t,y)] = e_k[x,y] + mask (1-bank tile) ----
            e4 = psum_pool.tile([128, 512], F32, tag="e4")
            e4v = e4[:].rearrange("p (k t y) -> p k t y", k=4, t=2)
            # mask lands first (identity matmul, start=True resets bank)
            nc.tensor.matmul(
                e4[:].rearrange("p (k ty) -> p k ty", k=4),
                lhsT=ident[:],
                rhs=im8v[:, :, gp, :],
                start=True, stop=False)
            for l in range(4):
                t, u = l // 2, l % 2
                nc.tensor.matmul(
                    e4v[u * 64:(u + 1) * 64, :, t, :],
                    lhsT=cmb[:, gp * 256 + l * 64: gp * 256 + (l + 1) * 64],
                    rhs=wallv[:, gp, :, l, :],
                    start=False, stop=True,
                    tile_position=(0, u * 64))

            # ---- per-k select: ONE max-reduce over the k axis (PSUM) ----
            z = work_pool.tile([128, 128], FP16, tag="z")
            nc.vector.tensor_reduce(
                z[:],
                e4[:].rearrange("p (k ty) -> p ty k", k=4),
                mybir.AxisListType.X, ALU.max)

            # ---- leakyrelu (masked entries stay <= -38) then exp ----
            pz = work_pool.tile([128, 128], FP16, tag="pz")
            nc.scalar.activation(pz[:], z[:], ACTF.Prelu, alpha=0.2)
            wal = wal_pool.tile([128, 128], BF16, tag="wal")
            nc.scalar.activation(wal[:], pz[:], ACTF.Exp)
            wals.append(wal)

        # bridge the oct-boundary PE gap to hold the clock at full speed
        filler(3)
        prev = (g, wals, hh8)

    out_block(*prev)


def build_nc():
    nc = bacc.Bacc("TRN2", target_bir_lowering=False, debug=False)
    cmb_d = nc.dram_tensor("cmb", [OCTS, 128, CW], BF16,
                           kind="ExternalInput").ap()
    id_d = nc.dram_tensor("ident", [128, 128], FP8,
                          kind="ExternalInput").ap()
    out_d = nc.dram_tensor("out", [OCTS, 128, OWC], BF16,
                           kind="ExternalOutput").ap()
    with tile.TileContext(nc) as tc:
        _kernel_body(tc, cmb_d, id_d, out_d)
    nc.compile()
    return nc


def _octify(x):
    """[B//4, 128, W] -> [B//8, 128, 2*W] pairing consecutive quads."""
    q, p, w = x.shape
    return (x.reshape(q // 2, 2, p, w).transpose(0, 2, 1, 3)
            .reshape(q // 2, p, 2 * w))


def prep_inputs(hidden, adj, a):
    """Host-side packing: bf16/fp8 casts, fused transposed layouts."""
    bf = ml_dtypes.bfloat16
    f8 = ml_dtypes.float8_e4m3
    hidden = np.asarray(hidden, dtype=np.float32)
    adj = np.asarray(adj)
    a = np.asarray(a, dtype=np.float32)

    hb = hidden.astype(bf)                                   # [B, 64, 128]

    # hT_q[q, d, l*64+i] = hidden[4q+l, i, d]
    hTf = (hidden.transpose(0, 2, 1)
           .reshape(B // 4, 4, D, N)
           .transpose(0, 2, 1, 3)
           .reshape(B // 4, D, 4 * N))
    hT = hTf.astype(bf)

    # wall_q[q, d, k*256+l*64+j] = hidden[4q+l, j, d] * a[k, d]
    wall = (hTf[:, None, :, :] * a[None, :, :, None]).astype(bf)
    wall = (wall.transpose(0, 2, 1, 3)
            .reshape(B // 4, D, 4 * 4 * N))

    # hh_q[q, u*64+j, t*HHW + c] : hidden rows + ones col for batch 4q+2t+u
    hh = np.zeros((B, N, HHW), dtype=bf)
    hh[:, :, 0:D] = hb
    hh[:, :, D] = bf(1.0)
    hhq = (hh.reshape(B // 4, 2, 2, N, HHW)
           .transpose(0, 2, 3, 1, 4)
           .reshape(B // 4, 2 * N, 2 * HHW))

    # indm[oct, u*64+x, k*256+g'*128+t*64+y] = 0 if adj[b][y,x]==k+1 else MASK
    # with b = oct*8 + g'*4 + t*2 + u; shipped as raw fp8 bytes inside cmb
    adjT = adj.transpose(0, 2, 1)                            # [b, x, y]
    mk = np.where(
        adjT[:, None, :, :] == np.arange(1, 5)[None, :, None, None],
        np.float32(0.0), np.float32(MASK)).astype(f8)        # [b, k, x, y]
    mk = mk.reshape(B // 8, 2, 2, 2, K, N, N)                # [o,g',t,u,k,x,y]
    indm = np.ascontiguousarray(
        mk.transpose(0, 3, 5, 4, 1, 2, 6).reshape(B // 8, 128, 1024))

    cmb16 = np.concatenate([_octify(hT), _octify(wall), _octify(hhq)], axis=2)
    cmb = np.concatenate(
        [cmb16.view(np.uint8), indm.view(np.uint8)], axis=2).view(bf)
    cmb = np.ascontiguousarray(cmb)                          # [B//8, 128, CW]

    ident = np.ascontiguousarray(np.eye(128, dtype=f8))

    in_maps = []
    for c in range(NCORES):
        gsl = slice(c * OCTS, (c + 1) * OCTS)
        in_maps.append({"cmb": np.ascontiguousarray(cmb[gsl]),
                        "ident": ident})
    return in_maps


_NC_CACHE = {}


def run_device(hidden, adj, a, **spmd_kwargs):
    if "nc" not in _NC_CACHE:
        _NC_CACHE["nc"] = build_nc()
    nc = _NC_CACHE["nc"]
    in_maps = prep_inputs(hidden, adj, a)
    res = run_bass_kernel_spmd(nc, in_maps, list(range(NCORES)), **spmd_kwargs)
    # res[c]["out"]: [OCTS, 128, OWC]; [g, u*64+i, (gp,t)*129 + c]
    full = np.concatenate([res.results[c]["out"] for c in range(NCORES)],
                          axis=0)
    full = full.astype(np.float32)
    full = full.reshape(B // 8, 2, N, 2, 2, 129)             # [g, u, i, gp, t, c]
    num = full[..., 0:D]
    den = full[..., D:D + 1]
    outq = (num / den).transpose(0, 3, 4, 1, 2, 5)           # [g, gp, t, u, i, d]
    out = np.ascontiguousarray(outq.reshape(B, N, D))
    return out.astype(np.float32), res


def kernel(hidden, adj, a):
    out, _ = run_device(hidden, adj, a)
    return out


# revision 11
# speedup vs baseline: 1.1684x; 1.1613x over previous
"""Bass/Trainium2 kernel for nn_LocalAggregator (GNN message passing).

Math per batch b (hidden [64,128], adj [64,64] in {0..4}, a [4,128]):
    e_k[i,j] = leakyrelu_{0.2}( sum_d hidden[i,d]*hidden[j,d]*a[k,d] )
    alpha    = softmax_j( where(adj==k+1, e_k, -9e15) )
    out      = alpha @ hidden

Device strategy (8 cores, 64 batches/core, OCT = 8 batches/iter):
  - e_k is SYMMETRIC in (i,j): masking with the host-TRANSPOSED
    adjacency yields transposed attention weights directly.
  - w_all[d,(k,l,j)] = hT * a_k precomputed on HOST (memory-bound
    regime: ship it, don't burn vector cycles).
  - ONE fused input DMA per oct; the one-hot additive mask ships as
    fp8 {0,-192} bytes inside the bf16 tensor (bitcast view on SBUF)
    and is ADDED into the e-PSUM by an fp8 identity matmul
    (start=True) before the e-matmuls accumulate on top.  The per-k
    select then becomes a max-fold:
      z_sel = max_k (lrelu(e_k + m_k));  masked entries stay <= -38
    so exp(z) ~ 0 there.  This kills the exp/select/mul/sum chain:
      ACT per gp: one Prelu [128,512]->fp16, one Exp [128,128]
      DVE per gp: two max-folds;  one strided output evac per oct
  - everything runs per GP-half (4 batches) on 1-bank PSUM tiles for
    deep PE/ACT/DVE overlap; ones-column in hh makes the out-matmul
    emit the softmax denominator; normalization on HOST.
"""

import numpy as np
import ml_dtypes

from contextlib import ExitStack

import concourse.bass as bass
import concourse.tile as tile
from concourse import bacc, mybir
from concourse._compat import with_exitstack
from concourse.bass_utils import run_bass_kernel_spmd

BF16 = mybir.dt.bfloat16
FP16 = mybir.dt.float16
FP8 = mybir.dt.float8e4
F32 = mybir.dt.float32
ALU = mybir.AluOpType
ACTF = mybir.ActivationFunctionType

B, N, D, K = 512, 64, 128, 4
NCORES = 8
BPC = B // NCORES          # 64 batches per core
OCTS = BPC // 8            # 8 octs of 8 batches per core
HHW = 132                  # hidden cols + ones col + pad
MASK = -192.0              # additive mask; exact in fp8e4m3, exp() -> ~0
CW = 512 + 2048 + 528 + 512  # hT8 | wall8 | hh8 | indm(fp8 as bf16) = 3600
OWC = 4 * 129              # out tile cols: (num 128 | denom) x (gp,t)


@with_exitstack
def _kernel_body(ctx, tc, cmb_d, id_d, out_d):
    nc = tc.nc

    const_pool = ctx.enter_context(tc.tile_pool(name="const", bufs=1))
    in_pool = ctx.enter_context(tc.tile_pool(name="inp", bufs=6))
    work_pool = ctx.enter_context(tc.tile_pool(name="work", bufs=4))
    wal_pool = ctx.enter_context(tc.tile_pool(name="walp", bufs=6))
    psum_pool = ctx.enter_context(tc.tile_pool(name="psum", bufs=3, space="PSUM"))
    opsum_pool = ctx.enter_context(tc.tile_pool(name="opsum", bufs=2, space="PSUM"))
    scr_pool = ctx.enter_context(tc.tile_pool(name="scr", bufs=1, space="PSUM"))
    out_pool = ctx.enter_context(tc.tile_pool(name="outp", bufs=4))

    ident = const_pool.tile([128, 128], FP8, tag="ident")
    nc.sync.dma_start(out=ident[:], in_=id_d)

    # PE keep-warm: the tensor engine only reaches full clock after ~3us
    # of gap-free execution.  A scratch stream with no input deps bridges
    # the startup (DMA latency) and the per-oct gaps so the real matmuls
    # run at full rate.  Results are never read.
    fsrc = const_pool.tile([128, 512], FP8, tag="fsrc")
    nc.gpsimd.memset(fsrc[:], 0)
    scr = scr_pool.tile([128, 512], F32, tag="scr")

    def filler(n):
        for _ in range(n):
            nc.tensor.matmul(scr[:], lhsT=fsrc[:, 0:128], rhs=fsrc[:],
                             start=True, stop=True)

    filler(16)

    def out_block(g, wals, hh8):
        """out matmuls + evac + output DMA for oct g (runs 1 oct behind
        the e-chain so the PE never waits on the ACT/DVE chain)."""
        ops = opsum_pool.tile([128, 1024], F32, tag="ops")
        for gp in range(2):
            for l in range(4):
                t, u = l // 2, l % 2
                nc.tensor.matmul(
                    ops[u * 64:(u + 1) * 64,
                        gp * 512 + t * HHW: gp * 512 + (t + 1) * HHW],
                    lhsT=wals[gp][u * 64:(u + 1) * 64, t * 64:(t + 1) * 64],
                    rhs=hh8[u * 64:(u + 1) * 64,
                            gp * 264 + t * HHW: gp * 264 + (t + 1) * HHW],
                    start=True, stop=True,
                    tile_position=(u * 64, u * 64))
        # compact evac (num|den only), alternating DVE/ACT for balance
        osb = out_pool.tile([128, OWC], BF16, tag="osb")
        src = ops[:].rearrange("p (g q) -> p g q", g=2)[:, :, 0:264]
        src = src.rearrange("p g (t c) -> p g t c", t=2)[:, :, :, 0:129]
        dst = osb[:].rearrange("p (g t c) -> p g t c", g=2, t=2)
        if g % 2 == 0:
            nc.vector.tensor_copy(dst, src)
        else:
            nc.scalar.activation(dst, src, ACTF.Copy)
        nc.gpsimd.dma_start(out=out_d[g], in_=osb[:])

    prev = None
    for g in range(OCTS):
        # fused oct load: 0:512 hT8 [d,(g',l,i)] | 512:2560 wall8
        # [d,(g',k,l,j)] | 2560:3088 hh8 [(u,j),(g',t,c)] |
        # 3088:3600 fp8 mask bytes [(u,x),(k,g',t,y)]
        cmb = in_pool.tile([128, CW], BF16, tag="cmb")
        nc.sync.dma_start(out=cmb[:], in_=cmb_d[g])

        # lagged out-block first: its inputs are long since ready, so the
        # PE stays busy while this oct's DMA lands
        if prev is not None:
            out_block(*prev)

        wallv = cmb[:, 512:2560].rearrange("p (g k l j) -> p g k l j",
                                           g=2, k=4, l=4)
        hh8 = cmb[:, 2560:3088]
        im8v = cmb[:, 3088:CW].bitcast(FP8).rearrange(
            "p (k g ty) -> p k g ty", k=4, g=2)

        wals = []
        for gp in range(2):
            # ---- e4[(u,x), (k,t,y)] = e_k[x,y] + mask (1-bank tile) ----
            e4 = psum_pool.tile([128, 512], F32, tag="e4")
            e4v = e4[:].rearrange("p (k t y) -> p k t y", k=4, t=2)
            # mask lands first (identity matmul, start=True resets bank)
            nc.tensor.matmul(
                e4[:].rearrange("p (k ty) -> p k ty", k=4),
                lhsT=ident[:],
                rhs=im8v[:, :, gp, :],
                start=True, stop=False)
            for l in range(4):
                t, u = l // 2, l % 2
                nc.tensor.matmul(
                    e4v[u * 64:(u + 1) * 64, :, t, :],
                    lhsT=cmb[:, gp * 256 + l * 64: gp * 256 + (l + 1) * 64],
                    rhs=wallv[:, gp, :, l, :],
                    start=False, stop=True,
                    tile_position=(0, u * 64))

            # ---- per-k select: ONE max-reduce over the k axis (PSUM) ----
            z = work_pool.tile([128, 128], FP16, tag="z")
            nc.vector.tensor_reduce(
                z[:],
                e4[:].rearrange("p (k ty) -> p ty k", k=4),
                mybir.AxisListType.X, ALU.max)

            # ---- leakyrelu (masked entries stay <= -38) then exp ----
            pz = work_pool.tile([128, 128], FP16, tag="pz")
            nc.scalar.activation(pz[:], z[:], ACTF.Prelu, alpha=0.2)
            wal = wal_pool.tile([128, 128], BF16, tag="wal")
            nc.scalar.activation(wal[:], pz[:], ACTF.Exp)
            wals.append(wal)

        # bridge the oct-boundary PE gap to hold the clock at full speed
        filler(5)
        prev = (g, wals, hh8)

    out_block(*prev)


def build_nc():
    nc = bacc.Bacc("TRN2", target_bir_lowering=False, debug=False)
    cmb_d = nc.dram_tensor("cmb", [OCTS, 128, CW], BF16,
                           kind="ExternalInput").ap()
    id_d = nc.dram_tensor("ident", [128, 128], FP8,
                          kind="ExternalInput").ap()
    out_d = nc.dram_tensor("out", [OCTS, 128, OWC], BF16,
                           kind="ExternalOutput").ap()
    with tile.TileContext(nc) as tc:
        _kernel_body(tc, cmb_d, id_d, out_d)
    nc.compile()
    return nc


def _octify(x):
    """[B//4, 128, W] -> [B//8, 128, 2*W] pairing consecutive quads."""
    q, p, w = x.shape
    return (x.reshape(q // 2, 2, p, w).transpose(0, 2, 1, 3)
            .reshape(q // 2, p, 2 * w))


def prep_inputs(hidden, adj, a):
    """Host-side packing: bf16/fp8 casts, fused transposed layouts."""
    bf = ml_dtypes.bfloat16
    f8 = ml_dtypes.float8_e4m3
    hidden = np.asarray(hidden, dtype=np.float32)
    adj = np.asarray(adj)
    a = np.asarray(a, dtype=np.float32)

    hb = hidden.astype(bf)                                   # [B, 64, 128]

    # hT_q[q, d, l*64+i] = hidden[4q+l, i, d]
    hTf = (hidden.transpose(0, 2, 1)
           .reshape(B // 4, 4, D, N)
           .transpose(0, 2, 1, 3)
           .reshape(B // 4, D, 4 * N))
    hT = hTf.astype(bf)

    # wall_q[q, d, k*256+l*64+j] = hidden[4q+l, j, d] * a[k, d]
    wall = (hTf[:, None, :, :] * a[None, :, :, None]).astype(bf)
    wall = (wall.transpose(0, 2, 1, 3)
            .reshape(B // 4, D, 4 * 4 * N))

    # hh_q[q, u*64+j, t*HHW + c] : hidden rows + ones col for batch 4q+2t+u
    hh = np.zeros((B, N, HHW), dtype=bf)
    hh[:, :, 0:D] = hb
    hh[:, :, D] = bf(1.0)
    hhq = (hh.reshape(B // 4, 2, 2, N, HHW)
           .transpose(0, 2, 3, 1, 4)
           .reshape(B // 4, 2 * N, 2 * HHW))

    # indm[oct, u*64+x, k*256+g'*128+t*64+y] = 0 if adj[b][y,x]==k+1 else MASK
    # with b = oct*8 + g'*4 + t*2 + u; shipped as raw fp8 bytes inside cmb
    adjT = adj.transpose(0, 2, 1)                            # [b, x, y]
    mk = np.where(
        adjT[:, None, :, :] == np.arange(1, 5)[None, :, None, None],
        np.float32(0.0), np.float32(MASK)).astype(f8)        # [b, k, x, y]
    mk = mk.reshape(B // 8, 2, 2, 2, K, N, N)                # [o,g',t,u,k,x,y]
    indm = np.ascontiguousarray(
        mk.transpose(0, 3, 5, 4, 1, 2, 6).reshape(B // 8, 128, 1024))

    cmb16 = np.concatenate([_octify(hT), _octify(wall), _octify(hhq)], axis=2)
    cmb = np.concatenate(
        [cmb16.view(np.uint8), indm.view(np.uint8)], axis=2).view(bf)
    cmb = np.ascontiguousarray(cmb)                          # [B//8, 128, CW]

    ident = np.ascontiguousarray(np.eye(128, dtype=f8))

    in_maps = []
    for c in range(NCORES):
        gsl = slice(c * OCTS, (c + 1) * OCTS)
        in_maps.append({"cmb": np.ascontiguousarray(cmb[gsl]),
                        "ident": ident})
    return in_maps


_NC_CACHE = {}


def run_device(hidden, adj, a, **spmd_kwargs):
    if "nc" not in _NC_CACHE:
        _NC_CACHE["nc"] = build_nc()
    nc = _NC_CACHE["nc"]
    in_maps = prep_inputs(hidden, adj, a)
    res = run_bass_kernel_spmd(nc, in_maps, list(range(NCORES)), **spmd_kwargs)
    # res[c]["out"]: [OCTS, 128, OWC]; [g, u*64+i, (gp,t)*129 + c]
    full = np.concatenate([res.results[c]["out"] for c in range(NCORES)],
                          axis=0)
    full = full.astype(np.float32)
    full = full.reshape(B // 8, 2, N, 2, 2, 129)             # [g, u, i, gp, t, c]
    num = full[..., 0:D]
    den = full[..., D:D + 1]
    outq = (num / den).transpose(0, 3, 4, 1, 2, 5)           # [g, gp, t, u, i, d]
    out = np.ascontiguousarray(outq.reshape(B, N, D))
    return out.astype(np.float32), res


def kernel(hidden, adj, a):
    out, _ = run_device(hidden, adj, a)
    return out


# revision 12
# speedup vs baseline: 1.2288x; 1.0517x over previous
"""Bass/Trainium2 kernel for nn_LocalAggregator (GNN message passing).

Math per batch b (hidden [64,128], adj [64,64] in {0..4}, a [4,128]):
    e_k[i,j] = leakyrelu_{0.2}( sum_d hidden[i,d]*hidden[j,d]*a[k,d] )
    alpha    = softmax_j( where(adj==k+1, e_k, -9e15) )
    out      = alpha @ hidden

Device strategy (8 cores, 64 batches/core). Matmuls + PSUM run per
QUAD (4 batches) for deep pipelining; elementwise runs per OCT
(8 batches) to amortize per-op fixed costs:
  - e_k is SYMMETRIC in (i,j): masking with the host-TRANSPOSED
    adjacency yields transposed attention weights directly.
  - w_all[d,(k,l,j)] = hT * a_k precomputed on HOST, shipped in the
    single fused oct DMA (one dma_start per 8 batches).
  - e-matmuls write a STRIDED PSUM AP -> e4 cols are (k,t,j); Prelu
    scatters quad halves into a (k, g', t, c) oct tile so Exp /
    one-hot eq / mask-mul / k-sum adds are contiguous full-width ops.
  - ones-column in hh makes the out-matmul emit the softmax
    denominator; normalization on HOST; f32->bf16 cast inside the
    SWDGE output DMA on the otherwise idle GpSimd engine.
"""

import numpy as np
import ml_dtypes

from contextlib import ExitStack

import concourse.bass as bass
import concourse.tile as tile
from concourse import bacc, mybir
from concourse._compat import with_exitstack
from concourse.bass_utils import run_bass_kernel_spmd

BF16 = mybir.dt.bfloat16
F32 = mybir.dt.float32
ALU = mybir.AluOpType
ACTF = mybir.ActivationFunctionType

B, N, D, K = 512, 64, 128, 4
NCORES = 8
BPC = B // NCORES          # 64 batches per core
QUADS = BPC // 4           # 16 quads of 4 batches per core
OCTS = BPC // 8            # 8 octs of 8 batches per core
HHW = 132                  # hidden cols + ones col + pad
# fused oct input cols: A = hT8 | wall8 ; B = adj8 | hh8
CWA = 512 + 2048                         # = 2560
CWB = 256 + 4 * HHW                      # = 784
OW = 2 * HHW               # out tile cols: (num 128 | denom | pad) x 2


@with_exitstack
def _kernel_body(ctx, tc, ina_d, inb_d, out_d):
    nc = tc.nc

    ina_pool = ctx.enter_context(tc.tile_pool(name="inpa", bufs=5))
    inb_pool = ctx.enter_context(tc.tile_pool(name="inpb", bufs=8))
    work_pool = ctx.enter_context(tc.tile_pool(name="work", bufs=8))
    psum_pool = ctx.enter_context(tc.tile_pool(name="psum", bufs=4, space="PSUM"))
    opsum_pool = ctx.enter_context(tc.tile_pool(name="opsum", bufs=2, space="PSUM"))
    out_pool = ctx.enter_context(tc.tile_pool(name="outp", bufs=6))

    for g in range(OCTS):
        # ---- two fused oct loads ----
        # A (released after e-matmuls):
        #   0:512     hT8   [128=d, (g',l,i)]     hidden^T, 8 batches
        #   512:2560  wall8 [128=d, (g',k,l,j)]   hT * a_k
        # B (small; held to the out-matmuls):
        #   0:256     adj8  [128=(u,r), (g',t,c)] transposed adjacency
        #   256:784   hh8   [128=(u,j), (g',t,c)] hidden rows + ones col
        cmb = ina_pool.tile([128, CWA], BF16, tag="cmba")
        nc.sync.dma_start(out=cmb[:], in_=ina_d[g])
        cmbb = inb_pool.tile([128, CWB], BF16, tag="cmbb")
        nc.sync.dma_start(out=cmbb[:], in_=inb_d[g])
        adj8 = cmbb[:, 0:256]
        wall8v = cmb[:, 512:CWA].rearrange(
            "p (g k l j) -> p g k l j", g=2, k=4, l=4)

        # one-hot indicators depend only on the small B load: run them
        # early so DVE works during the matmul/Prelu window
        ind8 = work_pool.tile([128, 1024], BF16, tag="ind8")
        for k in range(K):
            nc.vector.tensor_scalar(
                ind8[:, k * 256: (k + 1) * 256], adj8, float(k + 1),
                None, ALU.is_equal)

        # oct-wide elementwise tiles, col layout (k, g', t, c)
        lr8 = work_pool.tile([128, 1024], F32, tag="lr8")
        lr8v = lr8[:].rearrange("p (k g tc) -> p k g tc", k=4, g=2)

        e4s = []
        for gp in range(2):
            # ---- e4[(u,i), (k,t,j)] : 4 matmuls (strided PSUM out) ----
            e4 = psum_pool.tile([128, 512], F32, tag="e4")
            e4v = e4[:].rearrange("p (k t j) -> p k t j", k=4, t=2)
            for l in range(4):
                t, u = l // 2, l % 2
                nc.tensor.matmul(
                    e4v[u * 64: (u + 1) * 64, :, t, :],
                    lhsT=cmb[:, gp * 256 + l * 64: gp * 256 + (l + 1) * 64],
                    rhs=wall8v[:, gp, :, l, :],
                    start=True, stop=True,
                    tile_position=(0, u * 64),
                )
            e4s.append(e4)
            # ---- leakyrelu evacuates PSUM into the oct tile ----
            nc.scalar.activation(
                lr8v[:, :, gp, :],
                e4[:].rearrange("p (k tc) -> p k tc", k=4),
                ACTF.Prelu, alpha=0.2)

        # ---- oct-wide: exp, one-hot select, k-sum ----
        xm8 = work_pool.tile([128, 1024], BF16, tag="xm8")
        nc.scalar.activation(xm8[:], lr8[:], ACTF.Exp)
        w8 = work_pool.tile([128, 1024], BF16, tag="w8")
        nc.vector.tensor_mul(w8[:], xm8[:], ind8[:])
        t2 = work_pool.tile([128, 512], BF16, tag="t2")
        nc.vector.tensor_tensor(t2[:], w8[:, 0:512], w8[:, 512:1024], ALU.add)
        wsum = work_pool.tile([128, 256], BF16, tag="wsum")
        nc.vector.tensor_tensor(wsum[:], t2[:, 0:256], t2[:, 256:512], ALU.add)

        # ---- out matmuls: ONE 2-bank PSUM tile per oct ----
        # (gp halves sit at the 512-col bank boundary: no MM crosses a bank)
        ops = opsum_pool.tile([128, 1024], F32, tag="ops")
        for gp in range(2):
            for l in range(4):
                t, u = l // 2, l % 2
                nc.tensor.matmul(
                    ops[u * 64: (u + 1) * 64,
                        gp * 512 + t * HHW: gp * 512 + (t + 1) * HHW],
                    lhsT=wsum[u * 64: (u + 1) * 64,
                              gp * 128 + t * 64: gp * 128 + (t + 1) * 64],
                    rhs=cmbb[u * 64: (u + 1) * 64,
                             256 + (gp * 2 + t) * HHW: 256 + (gp * 2 + t + 1) * HHW],
                    start=True, stop=True,
                    tile_position=(u * 64, u * 64),
                )
        # ---- single compact evac + single output DMA per oct ----
        osb = out_pool.tile([128, 2 * OW], BF16, tag="osb")
        nc.vector.tensor_copy(
            osb[:].rearrange("p (g c) -> p g c", g=2),
            ops[:].rearrange("p (g c) -> p g c", g=2)[:, :, 0:OW])
        nc.gpsimd.dma_start(out=out_d[g], in_=osb[:])


def build_nc():
    nc = bacc.Bacc("TRN2", target_bir_lowering=False, debug=False)
    ina_d = nc.dram_tensor("cmba", [OCTS, 128, CWA], BF16,
                           kind="ExternalInput").ap()
    inb_d = nc.dram_tensor("cmbb", [OCTS, 128, CWB], BF16,
                           kind="ExternalInput").ap()
    out_d = nc.dram_tensor("out", [OCTS, 128, 2 * OW], BF16,
                           kind="ExternalOutput").ap()
    with tile.TileContext(nc) as tc:
        _kernel_body(tc, ina_d, inb_d, out_d)
    nc.compile()
    return nc


def _octify(x):
    """[B//4, 128, W] -> [B//8, 128, 2*W] pairing consecutive quads."""
    q, p, w = x.shape
    return (x.reshape(q // 2, 2, p, w).transpose(0, 2, 1, 3)
            .reshape(q // 2, p, 2 * w))


def prep_inputs(hidden, adj, a):
    """Host-side packing: bf16 casts, fused transposed/interleaved layouts."""
    bf = ml_dtypes.bfloat16
    hidden = np.asarray(hidden, dtype=np.float32)
    adj = np.asarray(adj)
    a = np.asarray(a, dtype=np.float32)

    hb = hidden.astype(bf)                                   # [B, 64, 128]

    # hT_q[q, d, l*64+i] = hidden[4q+l, i, d]
    hTf = (hidden.transpose(0, 2, 1)
           .reshape(B // 4, 4, D, N)
           .transpose(0, 2, 1, 3)
           .reshape(B // 4, D, 4 * N))
    hT = hTf.astype(bf)

    # adjT_q[q, u*64+r, t*64+c] = adj[4q+2t+u][c, r]
    adjT = adj.transpose(0, 2, 1).astype(bf)
    adjTq = (adjT.reshape(B // 4, 2, 2, N, N)
             .transpose(0, 2, 3, 1, 4)
             .reshape(B // 4, 2 * N, 2 * N))

    # hh_q[q, u*64+j, t*HHW + c] : hidden rows + ones col for batch 4q+2t+u
    hh = np.zeros((B, N, HHW), dtype=bf)
    hh[:, :, 0:D] = hb
    hh[:, :, D] = bf(1.0)
    hhq = (hh.reshape(B // 4, 2, 2, N, HHW)
           .transpose(0, 2, 3, 1, 4)
           .reshape(B // 4, 2 * N, 2 * HHW))

    # w_all_q[q, d, (k,l,j)] = hT[q,d,(l,j)] * a[k,d]
    wall = (hTf[:, None, :, :] * a[None, :, :, None]).astype(bf)  # [q,k,d,(l,j)]
    wall = (wall.transpose(0, 2, 1, 3)
            .reshape(B // 4, D, 4 * 4 * N))

    cmba = np.ascontiguousarray(
        np.concatenate([_octify(hT), _octify(wall)], axis=2))
    cmbb = np.ascontiguousarray(
        np.concatenate([_octify(adjTq), _octify(hhq)], axis=2))

    in_maps = []
    for c in range(NCORES):
        gsl = slice(c * OCTS, (c + 1) * OCTS)
        in_maps.append({"cmba": np.ascontiguousarray(cmba[gsl]),
                        "cmbb": np.ascontiguousarray(cmbb[gsl])})
    return in_maps


_NC_CACHE = {}


def run_device(hidden, adj, a, **spmd_kwargs):
    if "nc" not in _NC_CACHE:
        _NC_CACHE["nc"] = build_nc()
    nc = _NC_CACHE["nc"]
    in_maps = prep_inputs(hidden, adj, a)
    res = run_bass_kernel_spmd(nc, in_maps, list(range(NCORES)), **spmd_kwargs)
    # res[c]["out"]: [OCTS, 128, 2*OW]; [g, u*64+i, (gp,t)*HHW + d]
    full = np.concatenate([res.results[c]["out"] for c in range(NCORES)], axis=0)
    full = full.astype(np.float32)
    full = full.reshape(B // 8, 2, N, 2, 2, HHW)             # [g, u, i, gp, t, c]
    num = full[..., 0:D]
    den = full[..., D:D + 1]
    outq = (num / den).transpose(0, 3, 4, 1, 2, 5)           # [g, gp, t, u, i, d]
    out = np.ascontiguousarray(outq.reshape(B, N, D))
    return out.astype(np.float32), res


def kernel(hidden, adj, a):
    out, _ = run_device(hidden, adj, a)
    return out

